# revision 4
# baseline (speedup 1.0000x reference)
"""Supervised contrastive loss (nn_Batch_CL) on 8 Trainium2 NeuronCores. v2.

Math (per the reference):
  x = l2_normalize(feature_embeds)            # [N, D]
  logits = (x @ x.T) / tau                    # tau = 0.1
  Z_i    = sum_{j != i} exp(logits[i, j])
  S_i    = sum_{j != i, l_j == l_i} logits[i, j]
  P_i    = |{j != i : l_j == l_i}|
  per_row_i = S_i / P_i - log Z_i   (if P_i > 0 else 0)
  loss = -sum(per_row) / n_valid

Distribution: rows sharded 8 ways (1024 rows/core); each core receives the
full feature matrix with ITS OWN rows permuted to the front (so lhsT row
chunks and the diagonal live in build-half 0). Host epilogue combines the
per-core [sum per_row, n_valid] pairs.

v2 schedule (ACT-saturating redesign):
  - The exp of the [1024 x 8192] logits block is the hard floor (~55us on
    ACT at 1 elem/cycle/lane); everything else is arranged to hide behind
    it.
  - PSUM carve: tag A [128,2048] (banks 0-3), tag B [128,1536] (banks 4-6),
    bank 7 holds the Msum^T accumulation chain + all epilogue matmuls.
    Per-row tiling alternates A/B; columns are walked in snake order so
    consecutive exp tiles strictly ping-pong between A and B. Full double
    buffering with 7 banks, zero slot contention.
  - exp+row-sum fused via activation(Exp, scale=10, accum_out) per tile.
  - class sums as ONE 64-matmul PE accumulation chain into bank 7:
    Msum^T[d,c] += xh_chunk^T @ onehot_chunk; F = x_m^T-block @ Msum^T
    gives per-(row,class) sums; a one-hot select + accum_out yields S_i.
  - x streamed in 1024-row halves: DMA -> square+reduce (DVE) ->
    Ln/Exp rsqrt (ACT, stays in the natural_log_exp table) -> scale+bf16
    (DVE) -> DMA-transpose into xT. Half 0 is split in two for latency.
  - exact diagonal from ||xh_i||^2 (DVE, matches the bf16 PE products),
    removed from Z (exp then subtract) and S (subtract raw).
"""

import numpy as np

N = 8192
D = 128
N_CORES = 8
ROWS_PER_CORE = N // N_CORES          # 1024
NCHUNK = N // 128                     # 64 chunks of 128 rows
NHALF = 8                             # build halves of 1024 rows
NOWN = ROWS_PER_CORE // 128           # 8 own row-chunks
NCLS = 33
INV_TAU = 10.0
WA = 2048                             # tag-A psum tile width (4 banks)
WB = 1536                             # tag-B psum tile width (3 banks)
NT = 5                                # exp tiles per row-chunk
DEBUG_OUTPUTS = False

# per-m exp tile layout: (letter, col0, col1); even/odd m alternate so the
# snake walk (t ascending, m snaking) strictly alternates A,B,A,B,...
TILES_EVEN = [("A", 0, 1024), ("B", 1024, 2560), ("A", 2560, 4608),
              ("B", 4608, 6144), ("A", 6144, 8192)]
TILES_ODD = [("B", 0, 1024), ("A", 1024, 3072), ("B", 3072, 4608),
             ("A", 4608, 6656), ("B", 6656, 8192)]

_NC = None

# ---------------------------------------------------------------------------
# Inlined workarounds (kernel.py must be self-contained).
#
# The local walrus build accepts at most ONE sync-wait command per
# instruction (any type). Tile's scheduler attaches several. Two fixes:
#   1. TileContext._drain_and_barrier is replaced so the exit drain's many
#      waits are split across single-wait nops.
#   2. split_multiwait(nc): post-pass that hoists extra sync waits from any
#      instruction onto injected same-engine EventSemaphore instructions
#      placed immediately before it (engines are in-order, so this is
#      semantically identical).
# ---------------------------------------------------------------------------

_nop_counter = [0]


def _split_drain_and_barrier(self, tick_clock, wait_clock):
    import bass_rust

    vec = tick_clock.global_clock  # VectorClock
    for proc in range(len(vec)):
        tickv = vec[proc]
        if tickv > 0:
            nop_inst = self.nc.sync.nop(nofuse=True)
            c = bass_rust.ScopedClock()
            c.require_at_least(None, proc, tickv)
            wait_clock.add_sem_waits(nop_inst.ins, c)
    self.nc.sync.drain()
    self.nc.all_engine_barrier()
    assert self.sems is not None
    popped = self.nc._tile_sem_poison_stack.pop()
    assert popped is self._sem_poison
    self.nc.clear_and_free_semaphores(list(self.sems.allocated().values()))
    self.nc.all_engine_barrier()


def _install_tile_patch():
    from concourse import tile as _tile

    _tile.TileContext._drain_and_barrier = _split_drain_and_barrier


def _split_multiwait(nc):
    """Hoist all-but-one sync wait from every instruction onto nops."""
    import concourse.mybir as mybir

    n_hoisted = 0
    for bb in nc.main_func.blocks:
        insns = bb.instructions
        out = []
        changed = False
        for ins in insns:
            si = ins.sync_info
            if si is not None and len(si.on_wait) > 1:
                waits = list(si.on_wait)
                for w in waits[:-1]:
                    _nop_counter[0] += 1
                    nop = mybir.InstEventSemaphore(
                        name=f"hoistnop-{_nop_counter[0]}",
                        engine=ins.engine,
                        sync_info=mybir.SyncInfo(on_wait=[w], on_update=[]),
                    )
                    out.append(nop)
                    n_hoisted += 1
                ins.sync_info = mybir.SyncInfo(
                    on_wait=[waits[-1]], on_update=list(si.on_update)
                )
                changed = True
            out.append(ins)
        if changed:
            bb.instructions = out
    return n_hoisted


def _install_ntff_hook():
    """Synthesize the antenv.axon_hooks module missing from this image so
    run_bass_kernel_spmd(trace=True) can NTFF-profile under axon."""
    import sys
    import types

    if "antenv.axon_hooks" in sys.modules:
        return True
    try:
        import antenv
        from trn_agent_boot.trn_boot import _ntff_profile_via_ctypes
    except ImportError:
        return False
    hook_box = [None]
    mod = types.ModuleType("antenv.axon_hooks")
    mod.set_axon_ntff_profile_hook = lambda h: hook_box.__setitem__(0, h)
    mod.get_axon_ntff_profile_hook = lambda: hook_box[0]
    sys.modules["antenv.axon_hooks"] = mod
    antenv.axon_hooks = mod
    hook = _ntff_profile_via_ctypes("/opt/axon/libaxon_pjrt.so")
    mod.set_axon_ntff_profile_hook(hook)
    return hook is not None


def _build_nc(split_waits=True):
    import concourse.bass as bass
    import concourse.mybir as mybir
    from concourse import tile
    from contextlib import ExitStack

    _install_tile_patch()

    f32 = mybir.dt.float32
    bf16 = mybir.dt.bfloat16
    Alu = mybir.AluOpType
    Act = mybir.ActivationFunctionType
    X = mybir.AxisListType.X

    nc = bass.Bass()
    x_dram = nc.dram_tensor("xperm", [N, D], bf16, kind="ExternalInput")
    lab_dram = nc.dram_tensor("labels_pc", [128, NCHUNK], f32, kind="ExternalInput")
    iota_dram = nc.dram_tensor("iota33", [128, NCLS], f32, kind="ExternalInput")
    out_dram = nc.dram_tensor("out", [128, 2 * NOWN], f32, kind="ExternalOutput")
    if DEBUG_OUTPUTS:
        dbg = {
            name: nc.dram_tensor(name, shape, f32, kind="ExternalOutput")
            for name, shape in [
                ("dbg_zpart", [128, NOWN * NT]),
                ("dbg_rawdiag", [128, NOWN]),
                ("dbg_pown", [128, NOWN]),
                ("dbg_sfull", [128, NOWN]),
                ("dbg_parts", [128, 2]),
            ]
        }

    with tile.TileContext(nc) as tc, ExitStack() as ctx:
        persist = ctx.enter_context(tc.tile_pool(name="persist", bufs=1))

        xT = persist.tile([128, N], bf16)                 # normalized, transposed
        O_bf = persist.tile([128, NCHUNK * NCLS], bf16)   # one-hot labels (PE rhs)
        labels_sb = persist.tile([128, NCHUNK], f32)
        labels2 = persist.tile([128, NCHUNK], f32)
        zero1 = persist.tile([128, 1], f32)
        one1 = persist.tile([128, 1], f32)
        iota_sb = persist.tile([128, NCLS], f32)
        # per-rsqrt-batch tiles: chunks [0:4),[4:8),[8:24),[24:40),[40:64)
        # separate tiles avoid whole-tile false RAW between batches
        SSQ_BATCHES = [(0, 4), (4, 8), (8, 24), (24, 40), (40, 64)]
        ssq_b = [persist.tile([128, b - a], f32, name=f"ssqb{k}")
                 for k, (a, b) in enumerate(SSQ_BATCHES)]
        lns_b = [persist.tile([128, b - a], f32, name=f"lnsb{k}")
                 for k, (a, b) in enumerate(SSQ_BATCHES)]
        rinv_b = [persist.tile([128, b - a], f32, name=f"rinvb{k}")
                  for k, (a, b) in enumerate(SSQ_BATCHES)]
        rinvbf_b = [persist.tile([128, b - a], bf16, name=f"rinvbfb{k}")
                    for k, (a, b) in enumerate(SSQ_BATCHES)]

        def ssq_slot(c):
            """(batch_idx, local_offset) for global chunk c."""
            for k, (a, b) in enumerate(SSQ_BATCHES):
                if a <= c < b:
                    return k, c - a
            raise AssertionError(c)
        Zpart = persist.tile([128, NOWN * NT], f32)       # accum outs, m*NT+t
        rawdiag = persist.tile([128, NOWN], f32)
        Mt_sb = persist.tile([128, NCLS], bf16)
        cnt_part = persist.tile([128, NCLS], f32)
        cnt_row = persist.tile([1, NCLS], f32)
        cnt_bcast = persist.tile([128, NCLS], f32)
        ones_f = persist.tile([128, 1], f32)
        ones_row = persist.tile([1, 128], f32)
        e_dumpA = persist.tile([128, WA], f32)            # ACT out scratch (unread)
        e_dumpB = persist.tile([128, WB], f32)
        dump33 = persist.tile([128, NCLS], f32)
        dump128 = persist.tile([128, 128], bf16)
        P_own = persist.tile([128, NOWN], f32)
        S_full = persist.tile([128, NOWN], f32)
        res_sb = persist.tile([1, 2], f32)

        Zrow = persist.tile([128, NOWN], f32)
        e_diag = persist.tile([128, NOWN], f32)
        Zexcl = persist.tile([128, NOWN], f32)
        lnZ = persist.tile([128, NOWN], f32)
        S_excl = persist.tile([128, NOWN], f32)
        P_pos = persist.tile([128, NOWN], f32)
        P_safe = persist.tile([128, NOWN], f32)
        P_inv = persist.tile([128, NOWN], f32)
        valid = persist.tile([128, NOWN], f32)
        t_sp = persist.tile([128, NOWN], f32)
        perrow = persist.tile([128, NOWN], f32)
        loss_parts = persist.tile([128, 2], f32)

        # ---------------- tiny prologue loads ----------------
        nc.gpsimd.dma_start(labels_sb[:], lab_dram[:])
        nc.gpsimd.dma_start(iota_sb[:], iota_dram[:])
        nc.vector.memset(ones_f[:], 1.0)
        nc.vector.memset(ones_row[:], 1.0)

        with (
            tc.tile_pool(name="ps", bufs=1, space="PSUM") as ps_pool,
            tc.tile_pool(name="build", bufs=2) as build_pool,
        ):
            # psum carve: creation order fixes banks — A 0-3, B 4-6, small 7.
            # "small" is hand-sliced: Msum^T chain [0:33), F [33:297),
            # cnt [297:330), cnt-bcast [330:363), PE-transpose staging
            # (bf16, bitcast) [364:428) and [428:492).
            _unusedA = ps_pool.tile([128, WA], f32, tag="A", name="psA0")
            _unusedB = ps_pool.tile([128, WB], f32, tag="B", name="psB0")
            sm = ps_pool.tile([128, 512], f32, tag="small", name="smallps")
            small_box = [sm]

            def small_ps():
                return small_box[0]

            xh_halves = [None] * NHALF
            xs_halves = [None] * NHALF
            msum_pending = []   # (h, i) chunk-matmuls awaiting emission
            msum_emitted = [0]  # count of Msum chunk-matmuls emitted (of 64)

            def emit_msum(nmax):
                mpsT = small_ps()[:, 0:NCLS]
                n = 0
                while msum_pending and n < nmax:
                    h, i = msum_pending[0]
                    if xh_halves[h] is None:
                        break
                    msum_pending.pop(0)
                    c = h * 8 + i
                    k = msum_emitted[0]
                    nc.tensor.matmul(
                        mpsT,
                        xh_halves[h][:, i * 128:(i + 1) * 128],
                        O_bf[:, c * NCLS:(c + 1) * NCLS],
                        start=(k == 0),
                        stop=(k == NCHUNK - 1),
                    )
                    msum_emitted[0] += 1
                    n += 1

            def emit_load(h_lo, h_hi, parts=1):
                """One DMA for halves [h_lo, h_hi) into a fresh xs tile."""
                rows0, rows1 = h_lo * 1024, h_hi * 1024
                xs = build_pool.tile(
                    [128, rows1 - rows0], bf16, tag=f"xs{h_lo}",
                    name=f"xs{h_lo}", bufs=1)
                for q in range(parts):
                    n = (rows1 - rows0) // parts
                    nc.sync.dma_start(
                        xs[:, q * n:(q + 1) * n].rearrange(
                            "p (c d) -> p c d", d=128),
                        x_dram[rows0 + q * n:rows0 + (q + 1) * n, :].rearrange(
                            "(c p) d -> p c d", p=128),
                    )
                for h in range(h_lo, h_hi):
                    xs_halves[h] = xs[:, (h - h_lo) * 1024:(h - h_lo + 1) * 1024]

            def emit_norm(h, gate=1.0):
                """Row sum-of-squares for half h into ssq_all (DVE, fused).
                gate: 1.0 or a [128,1] AP holding 1.0 written late, used to
                keep the greedy list scheduler from slotting this ahead of
                the half-0 critical chain."""
                xs = xs_halves[h]
                for i in range(8):
                    k, off = ssq_slot(h * 8 + i)
                    nc.vector.scalar_tensor_tensor(
                        out=dump128[:],
                        in0=xs[:, i * 128:(i + 1) * 128],
                        scalar=gate,
                        in1=xs[:, i * 128:(i + 1) * 128],
                        op0=Alu.mult,
                        op1=Alu.mult,
                        accum_out=ssq_b[k][:, off:off + 1],
                    )

            def emit_rsqrt(a, b):
                """rinv = 1/sqrt(ssq) for chunk batch [a, b) via Ln/Exp."""
                k, off = ssq_slot(a)
                assert SSQ_BATCHES[k] == (a, b)
                nc.scalar.activation(lns_b[k][:], ssq_b[k][:], Act.Ln)
                nc.scalar.activation(
                    rinv_b[k][:], lns_b[k][:], Act.Exp, scale=-0.5)
                nc.vector.tensor_copy(rinvbf_b[k][:], rinv_b[k][:])

            def emit_scale(h, parts=1, pe=False):
                """xh = xs * rinv (bf16); transpose into xT (DMA or PE)."""
                rows0 = h * 1024
                xs = xs_halves[h]
                xh = build_pool.tile([128, 1024], bf16, tag=f"xh{h % 2}")
                np_ = 8 // parts
                for q in range(parts):
                    cl, ch_ = q * np_, (q + 1) * np_
                    nc.vector.scalar_tensor_tensor(
                        out=xh[:, cl * 128:ch_ * 128].rearrange(
                            "p (c r) -> p c r", r=128),
                        in0=xs[:, cl * 128:ch_ * 128].rearrange(
                            "p (c r) -> p c r", r=128),
                        scalar=1.0,
                        in1=rinvbf_b[ssq_slot(h * 8 + cl)[0]][
                            :, ssq_slot(h * 8 + cl)[1]:
                            ssq_slot(h * 8 + ch_ - 1)[1] + 1].to_broadcast(
                            (128, np_, 128)),
                        op0=Alu.mult,
                        op1=Alu.mult,
                    )
                    if pe:
                        # PE transpose via identity, staged in bank 7 (the
                        # F region, temporally disjoint); skips the DMA ring
                        # (whose queue is draining x loads)
                        for i in range(cl, ch_):
                            tp = sm[:, 33 + (i % 3) * 64:97 + (i % 3) * 64]
                            tp = tp.bitcast(bf16)
                            nc.tensor.transpose(
                                tp, xh[:, i * 128:(i + 1) * 128], eye_bf[:])
                            nc.vector.tensor_copy(
                                xT[:, rows0 + i * 128:rows0 + (i + 1) * 128],
                                tp)
                    else:
                        nc.sync.dma_start_transpose(
                            xT[:, rows0 + cl * 128:rows0 + ch_ * 128].rearrange(
                                "p (c r) -> p c r", r=128),
                            xh[:, cl * 128:ch_ * 128],
                        )
                xh_halves[h] = xh
                msum_pending.extend((h, i) for i in range(8))

            def emit_exp_tile(m, t):
                letter, c0, c1 = (TILES_EVEN if m % 2 == 0 else TILES_ODD)[t]
                w = c1 - c0
                ps = ps_pool.tile(
                    [128, WA if letter == "A" else WB], f32, tag=letter)
                lhsT = xT[:, m * 128:(m + 1) * 128]
                for k in range(w // 512):
                    nc.tensor.matmul(
                        ps[:, k * 512:(k + 1) * 512],
                        lhsT,
                        xT[:, c0 + k * 512:c0 + (k + 1) * 512],
                        start=True, stop=True,
                    )
                e_dump = e_dumpA if letter == "A" else e_dumpB
                nc.scalar.activation(
                    e_dump[:, 0:w], ps[:, 0:w], Act.Exp, scale=INV_TAU,
                    accum_out=Zpart[:, m * NT + t:m * NT + t + 1],
                )

            # ---- builds: 4 loads, rsqrt in 3 batches, scales ----
            emit_load(0, 1, parts=2)
            emit_norm(0)
            emit_rsqrt(0, 4)
            emit_rsqrt(4, 8)
            emit_scale(0, parts=2)
            emit_load(1, 3)
            with tc.tile_wait_until(0.017):
                emit_load(3, 5)
            with tc.tile_wait_until(0.022):
                emit_load(5, 8)
            # sim-time hints: keep the DVE free for the half-0 critical
            # chain first, then the halves feeding the next exp columns
            with tc.tile_wait_until(0.007):
                for h in (1, 2):
                    emit_norm(h)
                emit_rsqrt(8, 24)
                for h in (1, 2):
                    emit_scale(h)
            with tc.tile_wait_until(0.016):
                emit_norm(3)
                emit_norm(4)
                emit_rsqrt(24, 40)
                emit_scale(3)
                emit_scale(4)
            with tc.tile_wait_until(0.019):
                for h in (5, 6, 7):
                    emit_norm(h)
                emit_rsqrt(40, 64)
                for h in (5, 6, 7):
                    emit_scale(h)
            # label one-hot (DVE; needed from the t0 column onward)
            with tc.tile_wait_until(0.020):
                nc.vector.tensor_tensor(
                    out=O_bf[:].rearrange("p (c k) -> p c k", k=NCLS),
                    in0=iota_sb[:].rearrange("p (a k) -> p a k", a=1)
                    .to_broadcast((128, NCHUNK, NCLS)),
                    in1=labels_sb[:].to_broadcast((128, NCHUNK, NCLS)),
                    op=Alu.is_equal,
                )
            # rawdiag = ||xh_i||^2 for own rows (matches PE bf16 products)
            xh0 = xh_halves[0]
            tc.tile_set_cur_wait(0.030)
            for i in range(8):
                nc.vector.scalar_tensor_tensor(
                    out=dump128[:],
                    in0=xh0[:, i * 128:(i + 1) * 128],
                    scalar=1.0,
                    in1=xh0[:, i * 128:(i + 1) * 128],
                    op0=Alu.mult,
                    op1=Alu.mult,
                    accum_out=rawdiag[:, i:i + 1],
                )
            tc.tile_set_cur_wait(0)

            # ---- exp columns (snake), Msum matmuls drip-fed to PE ----
            # Msum matmuls are dripped at column turns: they fill the PE
            # idle window while the next column waits for its psum buffer,
            # keeping the PE p-state warm across the boundary
            for m in range(NOWN):
                emit_exp_tile(m, 0)
            emit_msum(24)
            for m in range(NOWN - 1, -1, -1):
                emit_exp_tile(m, 1)
            emit_msum(16)
            for m in range(NOWN):
                emit_exp_tile(m, 2)
            emit_msum(16)
            for m in range(NOWN - 1, -1, -1):
                emit_exp_tile(m, 3)
            emit_msum(64)  # flush any stragglers before the epilogue
            assert msum_emitted[0] == NCHUNK
            with tc.tile_wait_until(0.045):
                nc.vector.reduce_sum(
                    cnt_part[:],
                    O_bf[:].rearrange("p (c k) -> p k c", k=NCLS), axis=X)
            # diagonal exp terms: early, off the critical tail
            nc.scalar.activation(e_diag[:], rawdiag[:], Act.Exp, scale=INV_TAU)

            FB = NCLS                      # F block base: [FB, FB + 8*33)
            CB = FB + NOWN * NCLS          # cnt row base
            BB = CB + NCLS                 # cnt broadcast base

            for m in range(4):
                emit_exp_tile(m, 4)

            # ---- epilogue PE + select chains (overlap the t4 exps) ----
            # class counts: [1,33] then broadcast to [128,33]
            nc.tensor.matmul(
                sm[0:1, CB:CB + NCLS], ones_f[:], cnt_part[:],
                start=True, stop=True)
            nc.vector.tensor_copy(cnt_row[:], sm[0:1, CB:CB + NCLS])
            nc.tensor.matmul(
                sm[:, BB:BB + NCLS], ones_row[:], cnt_row[:],
                start=True, stop=True)
            nc.vector.tensor_copy(cnt_bcast[:], sm[:, BB:BB + NCLS])
            for m in range(NOWN):
                nc.vector.scalar_tensor_tensor(
                    out=dump33[:],
                    in0=O_bf[:, m * NCLS:(m + 1) * NCLS],
                    scalar=1.0,
                    in1=cnt_bcast[:],
                    op0=Alu.mult,
                    op1=Alu.mult,
                    accum_out=P_own[:, m:m + 1],
                )
            # Msum^T drain (chain stopped by the 64th chunk matmul)
            nc.vector.tensor_copy(Mt_sb[:], sm[:, 0:NCLS])
            for m in range(NOWN):
                nc.tensor.matmul(
                    sm[:, FB + m * NCLS:FB + (m + 1) * NCLS],
                    xT[:, m * 128:(m + 1) * 128],
                    Mt_sb[:],
                    start=True, stop=True,
                )
            for m in range(NOWN):
                nc.vector.scalar_tensor_tensor(
                    out=dump33[:],
                    in0=sm[:, FB + m * NCLS:FB + (m + 1) * NCLS],
                    scalar=1.0,
                    in1=O_bf[:, m * NCLS:(m + 1) * NCLS],
                    op0=Alu.mult,
                    op1=Alu.mult,
                    accum_out=S_full[:, m:m + 1],
                )
            # P / S assembly (independent of Z; overlaps t4)
            nc.vector.tensor_sub(S_excl[:], S_full[:], rawdiag[:])
            nc.vector.tensor_scalar_add(P_pos[:], P_own[:], -1.0)
            nc.vector.tensor_scalar_max(P_safe[:], P_pos[:], 1.0)
            nc.vector.reciprocal(P_inv[:], P_safe[:])
            nc.vector.tensor_scalar_min(valid[:], P_pos[:], 1.0)
            nc.vector.scalar_tensor_tensor(
                out=t_sp[:], in0=S_excl[:], scalar=INV_TAU, in1=P_inv[:],
                op0=Alu.mult, op1=Alu.mult,
            )

            def emit_perrow(ma, mb):
                s = slice(ma, mb)
                w = slice(ma * NT, mb * NT)
                nc.vector.reduce_sum(
                    Zrow[:, s],
                    Zpart[:, w].rearrange("p (m t) -> p m t", m=mb - ma),
                    axis=X)
                nc.vector.tensor_sub(Zexcl[:, s], Zrow[:, s], e_diag[:, s])
                nc.scalar.activation(lnZ[:, s], Zexcl[:, s], Act.Ln)
                nc.vector.tensor_sub(perrow[:, s], t_sp[:, s], lnZ[:, s])
                nc.vector.tensor_mul(perrow[:, s], perrow[:, s], valid[:, s])

            emit_perrow(0, 4)
            for m in range(4, NOWN):
                emit_exp_tile(m, 4)
            emit_perrow(4, NOWN)

            nc.sync.dma_start(out_dram[:, 0:NOWN], perrow[:])
            nc.sync.dma_start(out_dram[:, NOWN:2 * NOWN], valid[:])
            if DEBUG_OUTPUTS:
                nc.sync.dma_start(dbg["dbg_zpart"][:], Zpart[:])
                nc.sync.dma_start(dbg["dbg_rawdiag"][:], rawdiag[:])
                nc.sync.dma_start(dbg["dbg_pown"][:], P_own[:])
                nc.sync.dma_start(dbg["dbg_sfull"][:], S_full[:])
                nc.sync.dma_start(dbg["dbg_parts"][:], loss_parts[:])

    if split_waits:
        _split_multiwait(nc)
    return nc


def _get_nc(split_waits=True):
    global _NC
    if _NC is None:
        _NC = _build_nc(split_waits)
    return _NC


def _bf16():
    import ml_dtypes

    return ml_dtypes.bfloat16


def _make_in_maps(x, lab):
    iota = np.ascontiguousarray(
        np.tile(np.arange(NCLS, dtype=np.float32), (128, 1))
    )
    in_maps = []
    for c in range(N_CORES):
        lo, hi = c * ROWS_PER_CORE, (c + 1) * ROWS_PER_CORE
        perm = np.concatenate(
            [np.arange(lo, hi), np.arange(0, lo), np.arange(hi, N)]
        )
        xp = np.ascontiguousarray(x[perm].astype(_bf16()))
        lp = np.ascontiguousarray(
            lab[perm].astype(np.float32).reshape(NCHUNK, 128).T
        )
        in_maps.append({"xperm": xp, "labels_pc": lp, "iota33": iota})
    return in_maps


def _combine(results):
    parts = np.stack([np.asarray(results[c]["out"]) for c in range(N_CORES)])
    loss = (-parts[:, :, 0:NOWN].sum(dtype=np.float64)
            / parts[:, :, NOWN:2 * NOWN].sum(dtype=np.float64))
    return np.array(loss, dtype=np.float32)


def kernel(feature_embeds, label_ids):
    from concourse.bass_utils import run_bass_kernel_spmd

    x = np.asarray(feature_embeds, dtype=np.float32)
    lab = np.asarray(label_ids)
    nc = _get_nc()
    res = run_bass_kernel_spmd(nc, _make_in_maps(x, lab), list(range(N_CORES)))
    return _combine(res.results)


def kernel_profiled(feature_embeds, label_ids):
    """Same as kernel(), but with NTFF tracing; returns (loss, exec_time_ns)."""
    print("ntff hook installed:", _install_ntff_hook())
    from concourse.bass_utils import run_bass_kernel_spmd

    x = np.asarray(feature_embeds, dtype=np.float32)
    lab = np.asarray(label_ids)
    nc = _get_nc()
    res = run_bass_kernel_spmd(
        nc, _make_in_maps(x, lab), list(range(N_CORES)), trace=True
    )
    return _combine(res.results), res.exec_time_ns
